# revision 14
# baseline (speedup 1.0000x reference)
"""Multi-head attention (sparse/causal+valid_len) Bass kernel for TRN2.

Sharding: 8 cores = 2 batches x 4 head-groups (4 heads each).
Each core: this batch's x-tensors + its head-group's weight slices,
computes a partial (S, D_MODEL) output (its heads' contribution through
w_o); host sums the 4 partials per batch and adds biases.

Layouts (bf16 compute, fp32 accumulate in PSUM):
  qT, kT  [head_dim(2 heads=128), S]  - transposed projections
  v4      [k, h, (hi,lo), 65] fp8e4   - exact fp8 pair + ones col (denom)
  scoresT [k, q] in PSUM -> exp on ACT (scale=1/8, bias=vmask-shift) -> fp8
  causal zeroing of diagonal tiles via gpsimd affine_select
  attnV   fp8 DoubleRow: dup(exp) x (v_hi, v_lo) -> po4 [q, h, 65] fp32
  normalize: one scalar_tensor_tensor with broadcast reciprocals -> att bf16
  PE-transpose att -> attn_oT [hd, q] -> out-proj vs woT -> y fp16

The exp range shifts (per core, per 512-query block) keep exp() inside
fp8e4m3 dynamic range; softmax normalization cancels them exactly.
"""

import numpy as np
import ml_dtypes

import concourse.bass as bass
import concourse.mybir as mybir
import concourse.tile as tile
from concourse.masks import make_identity

BF16 = mybir.dt.bfloat16
FP16 = mybir.dt.float16
FP32 = mybir.dt.float32
FP8 = mybir.dt.float8e4

S = 2048
D = 1024
HEADS_PER_CORE = 4   # head-group size
DH = 64
HD = HEADS_PER_CORE * DH          # 256
NEG = -1.0e5                      # additive mask; exp underflows to exactly 0
NQB = S // 512

_MAX_WAITS = 1  # this container's walrus allows 1 sync wait per instruction

DR = mybir.MatmulPerfMode.DoubleRow


def fix_multi_waits(nc, max_waits: int = _MAX_WAITS):
    """Split >max_waits sem waits onto EventSemaphore insts placed just
    before the owning instruction (same engine => same semantics)."""
    import bass_rust
    n = 0
    for f in nc.m.functions:
        for bb in f.blocks:
            out = []
            changed = False
            for ins in bb.instructions:
                si = ins.sync_info
                waits = list(si.on_wait) if si is not None else []
                if len(waits) > max_waits:
                    changed = True
                    extra = waits[:-max_waits]
                    si.on_wait = waits[-max_waits:]
                    for i in range(0, len(extra), max_waits):
                        n += 1
                        es = mybir.InstEventSemaphore(
                            name=f"{ins.name}-esw{i}", ins=[], outs=[])
                        es.engine = ins.engine
                        es.sync_info = bass_rust.SyncInfo(
                            on_wait=extra[i:i + max_waits], on_update=[])
                        out.append(es)
                out.append(ins)
            if changed:
                bb.instructions = out
    return n


def build_kernel(KC: int, opts=None):
    opts = dict(opts or {})
    BIG_KC = KC > 8   # large valid_len: keep SBUF in budget
    EXP_BUFS = (2 * KC) if BIG_KC else (4 * KC + 2)
    PSCORE = opts.get("pscore", 2)
    PYO = opts.get("pyo", 2)
    PSMALL = opts.get("psmall", 2)
    ATTN_RATIO = opts.get("attn_ratio", 2)
    Y_ENG = opts.get("y_eng", "split")
    """Build the per-core Bass program. KC = number of 128-wide key chunks."""
    KP = KC * 128
    NQT = S // 128     # 16 query tiles of 128
    DM = D // 128      # 8 contraction chunks

    nc = bass.Bass()

    # DRAM I/O (per-core values supplied via in_maps)
    xqT_d = nc.dram_tensor("xqT", [D, S], BF16, kind="ExternalInput")
    xkT_d = nc.dram_tensor("xkT", [D, KP], BF16, kind="ExternalInput")
    xvT_d = nc.dram_tensor("xvT", [D, KP], BF16, kind="ExternalInput")
    wqT_d = nc.dram_tensor("wqT", [D, HD], BF16, kind="ExternalInput")
    wkT_d = nc.dram_tensor("wkT", [D, HD], BF16, kind="ExternalInput")
    wvT_d = nc.dram_tensor("wvT", [D, HD], BF16, kind="ExternalInput")
    woT_d = nc.dram_tensor("woT", [HD, D], BF16, kind="ExternalInput")
    vmask_d = nc.dram_tensor("vmask", [128, NQB * KC], FP32,
                             kind="ExternalInput")
    bqk_d = nc.dram_tensor("bqk", [128, 4], FP32, kind="ExternalInput")
    y_d = nc.dram_tensor("y", [S, D], FP16, kind="ExternalOutput")

    with tile.TileContext(nc) as tc:
        with (
            tc.tile_pool(name="const", bufs=1) as cpool,
            tc.tile_pool(name="win", bufs=1) as wpool,
            tc.tile_pool(name="qkv", bufs=1) as qkvpool,
            tc.tile_pool(name="ao", bufs=4) as aopool,
            tc.tile_pool(name="ysb", bufs=3) as ypool,
            tc.tile_pool(name="ps_score", bufs=PSCORE, space="PSUM") as pscore,
            tc.tile_pool(name="ps_yo", bufs=PYO, space="PSUM") as pyo,
            tc.tile_pool(name="ps_small", bufs=PSMALL, space="PSUM") as psmall,
        ):
            from contextlib import ExitStack
            xstack = ExitStack()
            xpool = xstack.enter_context(tc.tile_pool(name="xin", bufs=1))
            estack = ExitStack()
            epool = None
            if not BIG_KC:
                epool = estack.enter_context(
                    tc.tile_pool(name="expp", bufs=EXP_BUFS))
            # ---- constants (tiny, needed early) ----
            ident = cpool.tile([128, 128], BF16, tag="ident")
            make_identity(nc, ident[:, :])
            # ---- loads, ordered so scores(qb0) unblocks ASAP; kproj is
            # emitted c-outermost so PE chews each xk chunk as it lands ----
            wkT = wpool.tile([128, DM, HD], BF16, tag="wkT")
            wk_r = wkT_d[:].rearrange("(c p) f -> p c f", p=128)
            nc.sync.dma_start(wkT[:, 0:2, :], wk_r[:, 0:2, :])
            xkT = xpool.tile([128, DM, KP], BF16, tag="xkT")
            xk_r = xkT_d[:].rearrange("(c p) f -> p c f", p=128)
            nc.sync.dma_start(xkT[:, 0:2, :], xk_r[:, 0:2, :])
            bqk = cpool.tile([128, 4], FP32, tag="bqk")
            nc.sync.dma_start(bqk[:, :], bqk_d[:, :])
            vmask = cpool.tile([128, NQB * KC], FP32, tag="vmask")
            nc.sync.dma_start(vmask[:, :], vmask_d[:, :])
            nc.sync.dma_start(wkT[:, 2:DM, :], wk_r[:, 2:DM, :])
            for c in range(2, DM):
                nc.sync.dma_start(xkT[:, c, :], xk_r[:, c, :])
            wqT = wpool.tile([128, DM, HD], BF16, tag="wqT")
            nc.sync.dma_start(
                wqT[:, :, :], wqT_d[:].rearrange("(c p) f -> p c f", p=128))
            # xq per query-block (1MB each): scores(qb0) unblocks after the
            # first block; v-path loads overlap attention of qb0
            xqT = xpool.tile([128, DM, S], BF16, tag="xqT")
            xq_r = xqT_d[:].rearrange("(c p) f -> p c f", p=128)
            nc.sync.dma_start(xqT[:, :, 0:512], xq_r[:, :, 0:512])
            wvT = wpool.tile([128, DM, HD], BF16, tag="wvT")
            nc.sync.dma_start(
                wvT[:, :, :], wvT_d[:].rearrange("(c p) f -> p c f", p=128))
            xvT = xpool.tile([128, DM, KP], BF16, tag="xvT")
            xv_r = xvT_d[:].rearrange("(c p) f -> p c f", p=128)
            for c in range(0, DM, 4):
                nc.sync.dma_start(xvT[:, c:c + 4, :], xv_r[:, c:c + 4, :])
            for qs in range(512, S, 512):
                nc.sync.dma_start(
                    xqT[:, :, qs:qs + 512], xq_r[:, :, qs:qs + 512])
            woT = wpool.tile([128, 2, D], BF16, tag="woT")
            nc.sync.dma_start(
                woT[:, :, :], woT_d[:].rearrange("(c p) f -> p c f", p=128))

            # ---- K projection, c-outermost: accumulates each x-chunk as it
            # arrives; both ks-blocks' psum groups stay open in pscore ----
            kT = [qkvpool.tile([128, KP], BF16, tag=f"kT{j}", name=f"kT{j}")
                  for j in range(2)]
            KPROJ_IL = opts.get("kproj_interleave", True)
            if KPROJ_IL:
                # c-outermost: accumulates each x-chunk as it arrives; all
                # (block, j) psum groups stay open in pscore (distinct banks)
                kblocks = [(ks, min(512, KP - ks)) for ks in range(0, KP, 512)]
                kps = [pscore.tile([128, 2, 512], FP32, tag="pssc",
                                   name=f"kps{i}") for i in range(len(kblocks))]
                for c in range(DM):
                    for i, (ks, w) in enumerate(kblocks):
                        for j in range(2):
                            nc.tensor.matmul(
                                kps[i][:, j, :w],
                                wkT[:, c, 128 * j:128 * j + 128],
                                xkT[:, c, ks:ks + w],
                                start=(c == 0), stop=(c == DM - 1))
                for i, (ks, w) in enumerate(kblocks):
                    for j in range(2):
                        nc.vector.tensor_scalar_add(
                            kT[j][:, ks:ks + w], kps[i][:, j, :w],
                            bqk[:, 2 + j:3 + j])
            else:
                for ks in range(0, KP, 512):
                    for j in range(2):
                        w = min(512, KP - ks)
                        ps = pyo.tile([128, 512], FP32, tag="psy")
                        for c in range(DM):
                            nc.tensor.matmul(
                                ps[:, :w],
                                wkT[:, c, 128 * j:128 * j + 128],
                                xkT[:, c, ks:ks + w],
                                start=(c == 0), stop=(c == DM - 1))
                        nc.vector.tensor_scalar_add(
                            kT[j][:, ks:ks + w], ps[:, :w],
                            bqk[:, 2 + j:3 + j])

            # ---- V projection: generator, interleaved during qb0 scores.
            # Output as exact fp8 (hi, lo) pair + ones column (denominator).
            FP8_ATTN = opts.get("fp8_attn", False)
            ET_DT = FP8 if FP8_ATTN else BF16
            if FP8_ATTN:
                v_t = [qkvpool.tile([128, HEADS_PER_CORE, 2, 65], FP8,
                                    tag=f"v{kb}", name=f"v{kb}")
                       for kb in range(KC)]
            else:
                v_t = [qkvpool.tile([128, HEADS_PER_CORE, 65], BF16,
                                    tag=f"v{kb}", name=f"v{kb}")
                       for kb in range(KC)]

            def emit_vproj():
                for kb in range(KC):
                    vt = v_t[kb]
                    ps = pyo.tile([128, 512], FP32, tag="psy",
                                  name=f"psv{kb}")
                    for c in range(DM):
                        nc.tensor.matmul(
                            ps[:, :HD],
                            xvT[:, c, 128 * kb:128 * kb + 128],
                            wvT[:, c, :],
                            start=(c == 0), stop=(c == DM - 1))
                    psh = ps[:, :HD].rearrange("p (h e) -> p h e", e=64)
                    if FP8_ATTN:
                        nc.scalar.copy(vt[:, :, 0, 0:64], psh)
                        nc.vector.tensor_sub(vt[:, :, 1, 0:64], psh,
                                             vt[:, :, 0, 0:64])
                        nc.gpsimd.memset(vt[:, :, 0, 64:65], 1.0)
                        nc.gpsimd.memset(vt[:, :, 1, 64:65], 0.0)
                    else:
                        nc.vector.tensor_copy(vt[:, :, 0:64], psh)
                        nc.gpsimd.memset(vt[:, :, 64:65], 1.0)
                    yield

            # ---- Q projection (emitted per query block, pipelined) ----
            qT = [qkvpool.tile([128, S], BF16, tag=f"qT{j}", name=f"qT{j}")
                  for j in range(2)]

            def emit_qproj(qb):
                qs = 512 * qb
                for j in range(2):
                    ps = pyo.tile([128, 512], FP32, tag="psy",
                                  name=f"psq{qb}_{j}")
                    for c in range(DM):
                        nc.tensor.matmul(
                            ps[:, :],
                            wqT[:, c, 128 * j:128 * j + 128],
                            xqT[:, c, qs:qs + 512],
                            start=(c == 0), stop=(c == DM - 1))
                    nc.vector.tensor_scalar_add(
                        qT[j][:, qs:qs + 512], ps[:, :], bqk[:, j:j + 1])
                    yield

            # ---- attention + output projection, per 512-query block ----
            # software-pipelined: scores/exp for qb+1 are emitted before
            # attnV/outproj of qb so PE never waits on ACT's exp pass
            attn_oT = qkvpool.tile([128, 2, S], BF16, tag="aoT", name="aoT")
            exp_stage = {}
            epool_ref = [None]

            def emit_scores(qb):
                # generator: yields after each (kt, pair) score unit
                ktm = min(4 * qb + 3, KC - 1)   # causal+valid key-chunk bound
                # scoresT [k, q] -> exp (with per-qb range shift) -> fp8
                expT = [[None] * (ktm + 1) for _ in range(HEADS_PER_CORE)]
                exp_qlo = [0] * (ktm + 1)
                exp_stage[qb] = (expT, exp_qlo)
                for kt in range(ktm + 1):
                    for j in range(2):
                        # causal: queries below 128*kt never see this k chunk
                        qlo = max(0, 128 * kt - 512 * qb)
                        exp_qlo[kt] = qlo
                        w = 512 - qlo
                        # both row-halves (heads 2j, 2j+1) share one psum
                        # tile (different banks -> still concurrent on PE)
                        # and one exp + one causal-select instruction
                        ps = pscore.tile([128, 2, 512], FP32, tag="pssc",
                                         name=f"pssc{qb}_{kt}_{j}")
                        for r in range(2):
                            nc.tensor.matmul(
                                ps[:, r, :w],
                                kT[j][64 * r:64 * r + 64,
                                      128 * kt:128 * kt + 128],
                                qT[j][64 * r:64 * r + 64,
                                      512 * qb + qlo:512 * qb + 512],
                                start=True, stop=True)
                        et = epool_ref[0].tile([128, 2, w], ET_DT, tag="expT",
                                               name=f"expT{qb}_{kt}_{j}")
                        nc.scalar.activation(
                            et[:, :, :], ps[:, :, :w],
                            mybir.ActivationFunctionType.Exp,
                            bias=vmask[:, qb * KC + kt:qb * KC + kt + 1],
                            scale=0.125)
                        if 128 * kt + 127 > 512 * qb + qlo:
                            # zero strictly-above-diagonal: keep q >= k
                            # (r-dim coefficient 0: same mask per head)
                            nc.gpsimd.affine_select(
                                out=et[:, :, :], in_=et[:, :, :],
                                compare_op=mybir.AluOpType.is_ge,
                                fill=0.0,
                                base=512 * qb + qlo - 128 * kt,
                                pattern=[[0, 2], [1, w]],
                                channel_multiplier=-1)
                        expT[2 * j][kt] = et
                        expT[2 * j + 1][kt] = et
                        yield

            def emit_attn(qb):
                expT, exp_qlo = exp_stage.pop(qb)
                for qq in range(4):             # 128-query tiles in this block
                    qt = 4 * qb + qq
                    ktm_q = min(qt, KC - 1)
                    # all 4 heads share one psum bank: a single accumulation
                    # group (start zeroes the whole 2KB zero-region once)
                    po4 = psmall.tile([128, HEADS_PER_CORE, 65], FP32,
                                      tag="pso", name=f"po{qt}")
                    for h in range(HEADS_PER_CORE):
                        for kt in range(ktm_q + 1):
                            c0 = 128 * qq - exp_qlo[kt]
                            st = (h == 0 and kt == 0)
                            sp = (h == HEADS_PER_CORE - 1 and kt == ktm_q)
                            use_dr = opts.get("dr", True)
                            if isinstance(use_dr, list):
                                use_dr = qb in use_dr
                            if FP8_ATTN and use_dr:
                                lhsT = (expT[h][kt][:, h % 2, c0:c0 + 128]
                                        .unsqueeze(1)
                                        .broadcast_to([128, 2, 128]))
                                nc.tensor.matmul(
                                    po4[:, h, :], lhsT, v_t[kt][:, h, :, :],
                                    start=st, stop=sp,
                                    perf_mode=DR, skip_group_check=True)
                            elif FP8_ATTN:
                                es = expT[h][kt][:, h % 2, c0:c0 + 128]
                                nc.tensor.matmul(
                                    po4[:, h, :], es, v_t[kt][:, h, 0, :],
                                    start=st, stop=False,
                                    skip_group_check=True)
                                nc.tensor.matmul(
                                    po4[:, h, :], es, v_t[kt][:, h, 1, :],
                                    start=False, stop=sp,
                                    skip_group_check=True)
                            else:
                                nc.tensor.matmul(
                                    po4[:, h, :],
                                    expT[h][kt][:, h % 2, c0:c0 + 128],
                                    v_t[kt][:, h, :],
                                    start=st, stop=sp, skip_group_check=True)
                        yield
                    rec4 = aopool.tile([128, HEADS_PER_CORE], FP32, tag="rec")
                    for h in range(HEADS_PER_CORE):
                        nc.vector.reciprocal(rec4[:, h:h + 1],
                                             po4[:, h, 64:65])
                    att = aopool.tile([128, HD], BF16, tag="att",
                                      name=f"att{qt}")
                    nc.vector.scalar_tensor_tensor(
                        att[:].rearrange("p (h e) -> p h e", e=64),
                        po4[:, :, 0:64], 1.0,
                        rec4[:].unsqueeze(2).broadcast_to(
                            [128, HEADS_PER_CORE, 64]),
                        op0=mybir.AluOpType.mult, op1=mybir.AluOpType.mult)
                    # transpose att -> attn_oT (per head pair), evac on Pool
                    pst = psmall.tile([128, 2, 128], BF16, tag="pso",
                                      name=f"pst{qt}")
                    for j in range(2):
                        nc.tensor.matmul(
                            pst[:, j, :], att[:, 128 * j:128 * j + 128],
                            ident[:, :], is_transpose=True,
                            start=(j == 0), stop=(j == 1),
                            skip_group_check=True)
                    nc.vector.tensor_copy(
                        attn_oT[:, :, 128 * qt:128 * qt + 128],
                        pst[:, :, :])

                    # output projection for this query tile
                    ys = ypool.tile([128, D], FP16, tag="ysb")
                    for n in range(2):
                        ps = pyo.tile([128, 512], FP32, tag="psy")
                        for hc in range(2):
                            nc.tensor.matmul(
                                ps[:, :],
                                attn_oT[:, hc, 128 * qt:128 * qt + 128],
                                woT[:, hc, 512 * n:512 * n + 512],
                                start=(hc == 0), stop=(hc == 1))
                        # split PSUM evacuation between ACT and DVE
                        if Y_ENG == "act" or (Y_ENG == "split" and n == 0):
                            nc.scalar.copy(ys[:, 512 * n:512 * n + 512],
                                           ps[:, :])
                        else:
                            nc.vector.tensor_copy(
                                ys[:, 512 * n:512 * n + 512], ps[:, :])
                        nc.sync.dma_start(
                            y_d[128 * qt:128 * qt + 128,
                                512 * n:512 * n + 512],
                            ys[:, 512 * n:512 * n + 512])
                    yield

            if epool is not None:
                epool_ref[0] = epool
            # qb0's projection up front; later projections interleave one
            # block ahead of their scores
            for _ in emit_qproj(0):
                pass
            vp = emit_vproj()
            if BIG_KC:
                # all projections upfront, then release x inputs from SBUF
                # and only then open the (large) exp pool in the freed zone
                for _ in vp:
                    pass
                for qb_ in range(1, NQB):
                    for _ in emit_qproj(qb_):
                        pass
                vp = None
                xstack.close()
                epool_ref[0] = estack.enter_context(
                    tc.tile_pool(name="expp", bufs=EXP_BUFS))
            for qb in range(NQB + 1):
                sc = emit_scores(qb) if qb < NQB else None
                at = (emit_attn(qb - 1)
                      if qb >= 1 and qb - 1 in exp_stage else None)
                qp = (emit_qproj(qb + 1)
                      if (not BIG_KC and qb + 1 < NQB) else None)
                done_sc = sc is None
                done_at = at is None
                done_qp = qp is None
                if qb == 0 and vp is not None:
                    at, done_at = vp, False
                    vp = None
                while not (done_sc and done_at and done_qp):
                    if not done_sc:
                        try:
                            next(sc)
                        except StopIteration:
                            done_sc = True
                    if not done_at:
                        for _ in range(ATTN_RATIO):
                            try:
                                next(at)
                            except StopIteration:
                                done_at = True
                                break
                    if not done_qp:
                        try:
                            next(qp)
                        except StopIteration:
                            done_qp = True

            estack.close()
            if not BIG_KC:
                xstack.close()

    fix_multi_waits(nc)
    return nc


def prepare_inputs(inputs):
    """Host-side shard/cast/transpose. Returns (in_maps, KC, host_bias)."""
    f32 = np.float32
    xq = np.asarray(inputs["will_be_queries"], f32)
    xk = np.asarray(inputs["will_be_keys"], f32)
    xv = np.asarray(inputs["will_be_values"], f32)
    L = np.asarray(inputs["valid_len"]).astype(np.int64)
    w_q = np.asarray(inputs["w_q"], f32)
    w_k = np.asarray(inputs["w_k"], f32)
    w_v = np.asarray(inputs["w_v"], f32)
    w_o = np.asarray(inputs["w_o"], f32)
    b_q = np.asarray(inputs["b_q"], f32)
    b_k = np.asarray(inputs["b_k"], f32)
    b_o = np.asarray(inputs["b_o"], f32)
    b_v = np.asarray(inputs["b_v"], f32)

    B = xq.shape[0]
    Lmax = int(L.max())
    KC = (Lmax + 127) // 128
    KP = KC * 128
    bf = ml_dtypes.bfloat16

    def t_bf(a):  # (r, c) -> transposed bf16 contiguous
        return np.ascontiguousarray(a.T).astype(bf)

    bf16 = ml_dtypes.bfloat16
    in_maps = []
    for core in range(8):
        b, hg = divmod(core, 4)
        rows = slice(HD * hg, HD * hg + HD)
        # exp range shifts per q-block: exact block-max of valid logits
        # (device-identical bf16 q/k) keeps exp() inside fp8e4m3 range;
        # softmax normalization cancels the shift exactly.
        qTc = ((w_q[rows] @ xq[b].T).astype(bf16).astype(f32))  # (HD, S)
        kTc = ((w_k[rows] @ xk[b][:KP].T).astype(bf16).astype(f32))
        k_idx1 = np.arange(KP)[:, None]
        vm = np.full((128, KC), 0.0, f32)
        k_idx = (np.arange(KC)[None, :] * 128 + np.arange(128)[:, None])
        vm[k_idx >= L[b]] = NEG
        vm2 = np.empty((128, NQB * KC), f32)
        for qb in range(NQB):
            bmax, rmin = -1e9, 1e9
            q_idx1 = 512 * qb + np.arange(512)[None, :]
            for h in range(HEADS_PER_CORE):
                sc = (kTc[DH * h:DH * h + DH].T
                      @ qTc[DH * h:DH * h + DH, 512 * qb:512 * qb + 512])
                sc *= 0.125
                valid = (k_idx1 <= q_idx1) & (k_idx1 < L[b])
                scm = np.where(valid, sc, -1e9)
                rowmax = scm.max(0)
                has = valid.any(0)
                bmax = max(bmax, float(scm.max()))
                rmin = min(rmin, float(rowmax[has].min()))
            # exp(bmax-shift) <= e^4.9 = 134 < 240 (e4m3 max); keep the
            # weakest row's max above the subnormal flush threshold 2^-10
            shift = max(0.0, bmax - 4.9)
            shift = min(shift, rmin + 6.5)
            vm2[:, qb * KC:qb * KC + KC] = vm - shift
        bqk = np.zeros((128, 4), f32)
        bqk[:, 0] = b_q[rows][:128]
        bqk[:, 1] = b_q[rows][128:]
        bqk[:, 2] = b_k[rows][:128]
        bqk[:, 3] = b_k[rows][128:]
        in_maps.append({
            "xqT": t_bf(xq[b]),
            "xkT": t_bf(xk[b][:KP]),
            "xvT": t_bf(xv[b][:KP]),
            "wqT": t_bf(w_q[rows]),
            "wkT": t_bf(w_k[rows]),
            "wvT": t_bf(w_v[rows]),
            "woT": t_bf(w_o[:, rows]),
            "vmask": vm2,
            "bqk": bqk,
        })
    # exact host-side bias correction: y += b_o + w_o @ b_v
    host_bias = (b_o + w_o @ b_v).astype(f32)
    return in_maps, KC, host_bias


def combine_outputs(results, host_bias):
    B = 2
    out = np.zeros((B, S, D), np.float32)
    for core, res in enumerate(results):
        b = core // 4
        out[b] += np.asarray(res["y"], np.float32)
    out += host_bias[None, None, :]
    return out


# ---------------------------------------------------------------------------
# Harness entry point: full (unsharded) inputs -> full output.
# Shards across the 8 NeuronCores as 2 batches x 4 head-groups, runs the
# Bass kernel SPMD, and reduces the per-core partial outputs on the host.
# ---------------------------------------------------------------------------
def kernel(**inputs) -> np.ndarray:
    from concourse.bass_utils import run_bass_kernel_spmd

    in_maps, KC, host_bias = prepare_inputs(inputs)
    nc = build_kernel(KC, {"attn_ratio": 4})
    res = run_bass_kernel_spmd(nc, in_maps, list(range(8)))
    return combine_outputs(res.results, host_bias)


# revision 20
# speedup vs baseline: 1.0972x; 1.0972x over previous
"""Multi-head attention (sparse/causal+valid_len) Bass kernel for TRN2.

Sharding: 8 cores = 2 batches x 4 head-groups (4 heads each).
Each core: this batch's x-tensors + its head-group's weight slices,
computes a partial (S, D_MODEL) output (its heads' contribution through
w_o); host sums the 4 partials per batch and adds biases.

Layouts (bf16 compute, fp32 accumulate in PSUM):
  qT, kT  [head_dim(2 heads=128), S]  - transposed projections
  v4      [k, h, (hi,lo), 65] fp8e4   - exact fp8 pair + ones col (denom)
  scoresT [k, q] in PSUM -> exp on ACT (scale=1/8, bias=vmask-shift) -> fp8
  causal zeroing of diagonal tiles via gpsimd affine_select
  attnV   fp8 DoubleRow: dup(exp) x (v_hi, v_lo) -> po4 [q, h, 65] fp32
  normalize: one scalar_tensor_tensor with broadcast reciprocals -> att bf16
  PE-transpose att -> attn_oT [hd, q] -> out-proj vs woT -> y fp16

The exp range shifts (per core, per 512-query block) keep exp() inside
fp8e4m3 dynamic range; softmax normalization cancels them exactly.
"""

import numpy as np
import ml_dtypes

import concourse.bass as bass
import concourse.mybir as mybir
import concourse.tile as tile
from concourse.masks import make_identity

BF16 = mybir.dt.bfloat16
FP16 = mybir.dt.float16
FP32 = mybir.dt.float32
FP8 = mybir.dt.float8e4

S = 2048
D = 1024
HEADS_PER_CORE = 4   # head-group size
DH = 64
HD = HEADS_PER_CORE * DH          # 256
NEG = -1.0e5                      # additive mask; exp underflows to exactly 0
NQB = S // 512

_MAX_WAITS = 1  # this container's walrus allows 1 sync wait per instruction

DR = mybir.MatmulPerfMode.DoubleRow


def fix_multi_waits(nc, max_waits: int = _MAX_WAITS):
    """Split >max_waits sem waits onto EventSemaphore insts placed just
    before the owning instruction (same engine => same semantics)."""
    import bass_rust
    n = 0
    for f in nc.m.functions:
        for bb in f.blocks:
            out = []
            changed = False
            for ins in bb.instructions:
                si = ins.sync_info
                waits = list(si.on_wait) if si is not None else []
                if len(waits) > max_waits:
                    changed = True
                    extra = waits[:-max_waits]
                    si.on_wait = waits[-max_waits:]
                    for i in range(0, len(extra), max_waits):
                        n += 1
                        es = mybir.InstEventSemaphore(
                            name=f"{ins.name}-esw{i}", ins=[], outs=[])
                        es.engine = ins.engine
                        es.sync_info = bass_rust.SyncInfo(
                            on_wait=extra[i:i + max_waits], on_update=[])
                        out.append(es)
                out.append(ins)
            if changed:
                bb.instructions = out
    return n


def build_kernel(KC: int, opts=None):
    opts = dict(opts or {})
    BIG_KC = KC > 8   # large valid_len: keep SBUF in budget
    EXP_BUFS = (2 * KC) if BIG_KC else (4 * KC + 2)
    PSCORE = opts.get("pscore", 2)
    PYO = opts.get("pyo", 2)
    PSMALL = opts.get("psmall", 2)
    ATTN_RATIO = opts.get("attn_ratio", 2)
    Y_ENG = opts.get("y_eng", "split")
    """Build the per-core Bass program. KC = number of 128-wide key chunks."""
    KP = KC * 128
    NQT = S // 128     # 16 query tiles of 128
    DM = D // 128      # 8 contraction chunks

    nc = bass.Bass()

    # DRAM I/O (per-core values supplied via in_maps)
    xqT_d = nc.dram_tensor("xqT", [D, S], BF16, kind="ExternalInput")
    xkT_d = nc.dram_tensor("xkT", [D, KP], BF16, kind="ExternalInput")
    xvT_d = nc.dram_tensor("xvT", [D, KP], BF16, kind="ExternalInput")
    wqT_d = nc.dram_tensor("wqT", [D, HD], BF16, kind="ExternalInput")
    wkT_d = nc.dram_tensor("wkT", [D, HD], BF16, kind="ExternalInput")
    wvT_d = nc.dram_tensor("wvT", [D, HD], BF16, kind="ExternalInput")
    woT_d = nc.dram_tensor("woT", [HD, D], BF16, kind="ExternalInput")
    vmask_d = nc.dram_tensor("vmask", [128, NQB * KC], FP32,
                             kind="ExternalInput")
    bqk_d = nc.dram_tensor("bqk", [128, 4], FP32, kind="ExternalInput")
    y_d = nc.dram_tensor("y", [S, D], FP16, kind="ExternalOutput")

    with tile.TileContext(nc) as tc:
        with (
            tc.tile_pool(name="const", bufs=1) as cpool,
            tc.tile_pool(name="win", bufs=1) as wpool,
            tc.tile_pool(name="qkv", bufs=1) as qkvpool,
            tc.tile_pool(name="ao", bufs=4) as aopool,
            tc.tile_pool(name="ysb", bufs=3) as ypool,
            tc.tile_pool(name="ps_score", bufs=PSCORE, space="PSUM") as pscore,
            tc.tile_pool(name="ps_yo", bufs=PYO, space="PSUM") as pyo,
            tc.tile_pool(name="ps_small", bufs=PSMALL, space="PSUM") as psmall,
        ):
            from contextlib import ExitStack
            xstack = ExitStack()
            xpool = xstack.enter_context(tc.tile_pool(name="xin", bufs=1))
            estack = ExitStack()
            epool = None
            if not BIG_KC:
                epool = estack.enter_context(
                    tc.tile_pool(name="expp", bufs=EXP_BUFS))
            # ---- constants (tiny, needed early) ----
            ident = cpool.tile([128, 128], BF16, tag="ident")
            make_identity(nc, ident[:, :])
            # ---- loads, ordered so scores(qb0) unblocks ASAP; kproj is
            # emitted c-outermost so PE chews each xk chunk as it lands ----
            wkT = wpool.tile([128, DM, HD], BF16, tag="wkT")
            wk_r = wkT_d[:].rearrange("(c p) f -> p c f", p=128)
            nc.sync.dma_start(wkT[:, 0:2, :], wk_r[:, 0:2, :])
            xkT = xpool.tile([128, DM, KP], BF16, tag="xkT")
            xk_r = xkT_d[:].rearrange("(c p) f -> p c f", p=128)
            nc.sync.dma_start(xkT[:, 0:2, :], xk_r[:, 0:2, :])
            bqk = cpool.tile([128, 4], FP32, tag="bqk")
            nc.sync.dma_start(bqk[:, :], bqk_d[:, :])
            vmask = cpool.tile([128, NQB * KC], FP32, tag="vmask")
            nc.sync.dma_start(vmask[:, :], vmask_d[:, :])
            # q-side of block 0 loads before the remaining k chunks: qproj(0)
            # fills the PE while the rest of xk streams in
            wqT = wpool.tile([128, DM, HD], BF16, tag="wqT")
            nc.sync.dma_start(
                wqT[:, :, :], wqT_d[:].rearrange("(c p) f -> p c f", p=128))
            xqT = xpool.tile([128, DM, S], BF16, tag="xqT")
            xq_r = xqT_d[:].rearrange("(c p) f -> p c f", p=128)
            nc.sync.dma_start(xqT[:, :, 0:512], xq_r[:, :, 0:512])
            nc.sync.dma_start(wkT[:, 2:DM, :], wk_r[:, 2:DM, :])
            nc.sync.dma_start(xkT[:, 2:DM, :], xk_r[:, 2:DM, :])
            wvT = wpool.tile([128, DM, HD], BF16, tag="wvT")
            nc.sync.dma_start(
                wvT[:, :, :], wvT_d[:].rearrange("(c p) f -> p c f", p=128))
            xvT = xpool.tile([128, DM, KP], BF16, tag="xvT")
            xv_r = xvT_d[:].rearrange("(c p) f -> p c f", p=128)
            for c in range(0, DM, 4):
                nc.sync.dma_start(xvT[:, c:c + 4, :], xv_r[:, c:c + 4, :])
            for qs in range(512, S, 512):
                nc.sync.dma_start(
                    xqT[:, :, qs:qs + 512], xq_r[:, :, qs:qs + 512])
            woT = wpool.tile([128, 2, D], BF16, tag="woT")
            nc.sync.dma_start(
                woT[:, :, :], woT_d[:].rearrange("(c p) f -> p c f", p=128))

            # ---- K projection, c-outermost: accumulates each x-chunk as it
            # arrives; both ks-blocks' psum groups stay open in pscore ----
            kT = [qkvpool.tile([128, KP], BF16, tag=f"kT{j}", name=f"kT{j}")
                  for j in range(2)]
            # c-outermost: accumulates each x-chunk as it arrives; all
            # (block, j) psum groups stay open in pscore (distinct banks).
            # Emitted in two parts with qproj(0) between so the PE has work
            # while the trailing xk chunks stream in.
            kblocks = [(ks, min(512, KP - ks)) for ks in range(0, KP, 512)]
            kps = [pscore.tile([128, 2, 512], FP32, tag="pssc",
                               name=f"kps{i}") for i in range(len(kblocks))]

            def emit_kproj(cs):
                for c in cs:
                    for i, (ks, w) in enumerate(kblocks):
                        for j in range(2):
                            nc.tensor.matmul(
                                kps[i][:, j, :w],
                                wkT[:, c, 128 * j:128 * j + 128],
                                xkT[:, c, ks:ks + w],
                                start=(c == 0), stop=(c == DM - 1))
                if cs[-1] == DM - 1:
                    for i, (ks, w) in enumerate(kblocks):
                        for j in range(2):
                            nc.vector.tensor_scalar_add(
                                kT[j][:, ks:ks + w], kps[i][:, j, :w],
                                bqk[:, 2 + j:3 + j])

            # ---- V projection: generator, interleaved during qb0 scores.
            # Output as exact fp8 (hi, lo) pair + ones column (denominator).
            FP8_ATTN = opts.get("fp8_attn", False)
            ET_DT = FP8 if FP8_ATTN else BF16
            if FP8_ATTN:
                v_t = [qkvpool.tile([128, HEADS_PER_CORE, 2, 65], FP8,
                                    tag=f"v{kb}", name=f"v{kb}")
                       for kb in range(KC)]
            else:
                v_t = [qkvpool.tile([128, HEADS_PER_CORE, 65], BF16,
                                    tag=f"v{kb}", name=f"v{kb}")
                       for kb in range(KC)]

            def emit_vproj():
                for kb in range(KC):
                    vt = v_t[kb]
                    ps = pyo.tile([128, 512], FP32, tag="psy",
                                  name=f"psv{kb}")
                    for c in range(DM):
                        nc.tensor.matmul(
                            ps[:, :HD],
                            xvT[:, c, 128 * kb:128 * kb + 128],
                            wvT[:, c, :],
                            start=(c == 0), stop=(c == DM - 1))
                    psh = ps[:, :HD].rearrange("p (h e) -> p h e", e=64)
                    if FP8_ATTN:
                        nc.scalar.copy(vt[:, :, 0, 0:64], psh)
                        nc.vector.tensor_sub(vt[:, :, 1, 0:64], psh,
                                             vt[:, :, 0, 0:64])
                        nc.gpsimd.memset(vt[:, :, 0, 64:65], 1.0)
                        nc.gpsimd.memset(vt[:, :, 1, 64:65], 0.0)
                    else:
                        nc.vector.tensor_copy(vt[:, :, 0:64], psh)
                        nc.gpsimd.memset(vt[:, :, 64:65], 1.0)
                    yield

            # ---- Q projection (emitted per query block, pipelined) ----
            qT = [qkvpool.tile([128, S], BF16, tag=f"qT{j}", name=f"qT{j}")
                  for j in range(2)]

            def emit_qproj(qb):
                qs = 512 * qb
                for j in range(2):
                    ps = pyo.tile([128, 512], FP32, tag="psy",
                                  name=f"psq{qb}_{j}")
                    for c in range(DM):
                        nc.tensor.matmul(
                            ps[:, :],
                            wqT[:, c, 128 * j:128 * j + 128],
                            xqT[:, c, qs:qs + 512],
                            start=(c == 0), stop=(c == DM - 1))
                    nc.vector.tensor_scalar_add(
                        qT[j][:, qs:qs + 512], ps[:, :], bqk[:, j:j + 1])
                    yield

            # ---- attention + output projection, per 512-query block ----
            # software-pipelined: scores/exp for qb+1 are emitted before
            # attnV/outproj of qb so PE never waits on ACT's exp pass
            attn_oT = qkvpool.tile([128, 2, S], BF16, tag="aoT", name="aoT")
            exp_stage = {}
            epool_ref = [None]

            def emit_scores(qb):
                # generator: yields after each (kt, pair) score unit
                ktm = min(4 * qb + 3, KC - 1)   # causal+valid key-chunk bound
                # scoresT [k, q] -> exp (with per-qb range shift) -> fp8
                expT = [[None] * (ktm + 1) for _ in range(HEADS_PER_CORE)]
                exp_qlo = [0] * (ktm + 1)
                exp_stage[qb] = (expT, exp_qlo)
                for kt in range(ktm + 1):
                    for j in range(2):
                        # causal: queries below 128*kt never see this k chunk
                        qlo = max(0, 128 * kt - 512 * qb)
                        exp_qlo[kt] = qlo
                        w = 512 - qlo
                        # both row-halves (heads 2j, 2j+1) share one psum
                        # tile (different banks -> still concurrent on PE)
                        # and one exp + one causal-select instruction
                        ps = pscore.tile([128, 2, 512], FP32, tag="pssc",
                                         name=f"pssc{qb}_{kt}_{j}")
                        for r in range(2):
                            nc.tensor.matmul(
                                ps[:, r, :w],
                                kT[j][64 * r:64 * r + 64,
                                      128 * kt:128 * kt + 128],
                                qT[j][64 * r:64 * r + 64,
                                      512 * qb + qlo:512 * qb + 512],
                                start=True, stop=True)
                        et = epool_ref[0].tile([128, 2, w], ET_DT, tag="expT",
                                               name=f"expT{qb}_{kt}_{j}")
                        nc.scalar.activation(
                            et[:, :, :], ps[:, :, :w],
                            mybir.ActivationFunctionType.Exp,
                            bias=vmask[:, qb * KC + kt:qb * KC + kt + 1],
                            scale=0.125)
                        if 128 * kt + 127 > 512 * qb + qlo:
                            # zero strictly-above-diagonal: keep q >= k
                            # (r-dim coefficient 0: same mask per head)
                            nc.gpsimd.affine_select(
                                out=et[:, :, :], in_=et[:, :, :],
                                compare_op=mybir.AluOpType.is_ge,
                                fill=0.0,
                                base=512 * qb + qlo - 128 * kt,
                                pattern=[[0, 2], [1, w]],
                                channel_multiplier=-1)
                        expT[2 * j][kt] = et
                        expT[2 * j + 1][kt] = et
                        yield

            # 2-deep skewed attention pipeline: PE order per step is
            # attnV(qt), transposes(qt-1), outproj(qt-2) so DVE/ACT
            # evacuations of a tile overlap PE work on its neighbours
            stage_t = []   # (qt, att) awaiting transpose
            stage_o = []   # qt awaiting out-projection

            def flush_transpose():
                qt, att = stage_t.pop(0)
                pst = psmall.tile([128, 2, 128], BF16, tag="pso",
                                  name=f"pst{qt}")
                for j in range(2):
                    nc.tensor.matmul(
                        pst[:, j, :], att[:, 128 * j:128 * j + 128],
                        ident[:, :], is_transpose=True,
                        start=(j == 0), stop=(j == 1),
                        skip_group_check=True)
                nc.vector.tensor_copy(
                    attn_oT[:, :, 128 * qt:128 * qt + 128], pst[:, :, :])
                stage_o.append(qt)

            def flush_outproj():
                qt = stage_o.pop(0)
                ys = ypool.tile([128, D], FP16, tag="ysb")
                for n in range(2):
                    ps = pyo.tile([128, 512], FP32, tag="psy")
                    for hc in range(2):
                        nc.tensor.matmul(
                            ps[:, :],
                            attn_oT[:, hc, 128 * qt:128 * qt + 128],
                            woT[:, hc, 512 * n:512 * n + 512],
                            start=(hc == 0), stop=(hc == 1))
                    # split PSUM evacuation between ACT and DVE
                    if Y_ENG == "act" or (Y_ENG == "split" and n == 0):
                        nc.scalar.copy(ys[:, 512 * n:512 * n + 512],
                                       ps[:, :])
                    else:
                        nc.vector.tensor_copy(
                            ys[:, 512 * n:512 * n + 512], ps[:, :])
                    nc.sync.dma_start(
                        y_d[128 * qt:128 * qt + 128,
                            512 * n:512 * n + 512],
                        ys[:, 512 * n:512 * n + 512])

            def emit_attn(qb):
                expT, exp_qlo = exp_stage.pop(qb)
                for qq in range(4):             # 128-query tiles in this block
                    qt = 4 * qb + qq
                    ktm_q = min(qt, KC - 1)
                    # all 4 heads share one psum bank: a single accumulation
                    # group (start zeroes the whole 2KB zero-region once)
                    po4 = psmall.tile([128, HEADS_PER_CORE, 65], FP32,
                                      tag="pso", name=f"po{qt}")
                    for h in range(HEADS_PER_CORE):
                        for kt in range(ktm_q + 1):
                            c0 = 128 * qq - exp_qlo[kt]
                            st = (h == 0 and kt == 0)
                            sp = (h == HEADS_PER_CORE - 1 and kt == ktm_q)
                            use_dr = opts.get("dr", True)
                            if isinstance(use_dr, list):
                                use_dr = qb in use_dr
                            if FP8_ATTN and use_dr:
                                lhsT = (expT[h][kt][:, h % 2, c0:c0 + 128]
                                        .unsqueeze(1)
                                        .broadcast_to([128, 2, 128]))
                                nc.tensor.matmul(
                                    po4[:, h, :], lhsT, v_t[kt][:, h, :, :],
                                    start=st, stop=sp,
                                    perf_mode=DR, skip_group_check=True)
                            elif FP8_ATTN:
                                es = expT[h][kt][:, h % 2, c0:c0 + 128]
                                nc.tensor.matmul(
                                    po4[:, h, :], es, v_t[kt][:, h, 0, :],
                                    start=st, stop=False,
                                    skip_group_check=True)
                                nc.tensor.matmul(
                                    po4[:, h, :], es, v_t[kt][:, h, 1, :],
                                    start=False, stop=sp,
                                    skip_group_check=True)
                            else:
                                nc.tensor.matmul(
                                    po4[:, h, :],
                                    expT[h][kt][:, h % 2, c0:c0 + 128],
                                    v_t[kt][:, h, :],
                                    start=st, stop=sp, skip_group_check=True)
                        yield
                    rec4 = aopool.tile([128, HEADS_PER_CORE], FP32, tag="rec")
                    for h in range(HEADS_PER_CORE):
                        nc.vector.reciprocal(rec4[:, h:h + 1],
                                             po4[:, h, 64:65])
                    att = aopool.tile([128, HD], BF16, tag="att",
                                      name=f"att{qt}")
                    nc.vector.scalar_tensor_tensor(
                        att[:].rearrange("p (h e) -> p h e", e=64),
                        po4[:, :, 0:64], 1.0,
                        rec4[:].unsqueeze(2).broadcast_to(
                            [128, HEADS_PER_CORE, 64]),
                        op0=mybir.AluOpType.mult, op1=mybir.AluOpType.mult)
                    stage_t.append((qt, att))
                    if len(stage_t) > 1:
                        flush_transpose()
                    if len(stage_o) > 1:
                        flush_outproj()
                    yield

            if epool is not None:
                epool_ref[0] = epool
            # kproj(c0-1) -> qproj(0) -> kproj(c2-7): the PE chews early
            # chunks while the rest of xk/xq streams in
            emit_kproj(list(range(0, 2)))
            for _ in emit_qproj(0):
                pass
            emit_kproj(list(range(2, DM)))
            vp = emit_vproj()
            if BIG_KC:
                # all projections upfront, then release x inputs from SBUF
                # and only then open the (large) exp pool in the freed zone
                for _ in vp:
                    pass
                for qb_ in range(1, NQB):
                    for _ in emit_qproj(qb_):
                        pass
                vp = None
                xstack.close()
                epool_ref[0] = estack.enter_context(
                    tc.tile_pool(name="expp", bufs=EXP_BUFS))
            for qb in range(NQB + 1):
                sc = emit_scores(qb) if qb < NQB else None
                at = (emit_attn(qb - 1)
                      if qb >= 1 and qb - 1 in exp_stage else None)
                qp = (emit_qproj(qb + 1)
                      if (not BIG_KC and qb + 1 < NQB) else None)
                done_sc = sc is None
                done_at = at is None
                done_qp = qp is None
                if qb == 0 and vp is not None:
                    at, done_at = vp, False
                    vp = None
                while not (done_sc and done_at and done_qp):
                    if not done_sc:
                        try:
                            next(sc)
                        except StopIteration:
                            done_sc = True
                    if not done_at:
                        for _ in range(ATTN_RATIO):
                            try:
                                next(at)
                            except StopIteration:
                                done_at = True
                                break
                    if not done_qp:
                        try:
                            next(qp)
                        except StopIteration:
                            done_qp = True

            # drain the skewed attention pipeline
            while stage_t or stage_o:
                if stage_t:
                    flush_transpose()
                if stage_o:
                    flush_outproj()

            estack.close()
            if not BIG_KC:
                xstack.close()

    fix_multi_waits(nc)
    return nc


def prepare_inputs(inputs):
    """Host-side shard/cast/transpose. Returns (in_maps, KC, host_bias)."""
    f32 = np.float32
    xq = np.asarray(inputs["will_be_queries"], f32)
    xk = np.asarray(inputs["will_be_keys"], f32)
    xv = np.asarray(inputs["will_be_values"], f32)
    L = np.asarray(inputs["valid_len"]).astype(np.int64)
    w_q = np.asarray(inputs["w_q"], f32)
    w_k = np.asarray(inputs["w_k"], f32)
    w_v = np.asarray(inputs["w_v"], f32)
    w_o = np.asarray(inputs["w_o"], f32)
    b_q = np.asarray(inputs["b_q"], f32)
    b_k = np.asarray(inputs["b_k"], f32)
    b_o = np.asarray(inputs["b_o"], f32)
    b_v = np.asarray(inputs["b_v"], f32)

    B = xq.shape[0]
    Lmax = int(L.max())
    KC = (Lmax + 127) // 128
    KP = KC * 128
    bf = ml_dtypes.bfloat16

    def t_bf(a):  # (r, c) -> transposed bf16 contiguous
        return np.ascontiguousarray(a.T).astype(bf)

    bf16 = ml_dtypes.bfloat16
    in_maps = []
    for core in range(8):
        b, hg = divmod(core, 4)
        rows = slice(HD * hg, HD * hg + HD)
        # exp range shifts per q-block: exact block-max of valid logits
        # (device-identical bf16 q/k) keeps exp() inside fp8e4m3 range;
        # softmax normalization cancels the shift exactly.
        qTc = ((w_q[rows] @ xq[b].T).astype(bf16).astype(f32))  # (HD, S)
        kTc = ((w_k[rows] @ xk[b][:KP].T).astype(bf16).astype(f32))
        k_idx1 = np.arange(KP)[:, None]
        vm = np.full((128, KC), 0.0, f32)
        k_idx = (np.arange(KC)[None, :] * 128 + np.arange(128)[:, None])
        vm[k_idx >= L[b]] = NEG
        vm2 = np.empty((128, NQB * KC), f32)
        for qb in range(NQB):
            bmax, rmin = -1e9, 1e9
            q_idx1 = 512 * qb + np.arange(512)[None, :]
            for h in range(HEADS_PER_CORE):
                sc = (kTc[DH * h:DH * h + DH].T
                      @ qTc[DH * h:DH * h + DH, 512 * qb:512 * qb + 512])
                sc *= 0.125
                valid = (k_idx1 <= q_idx1) & (k_idx1 < L[b])
                scm = np.where(valid, sc, -1e9)
                rowmax = scm.max(0)
                has = valid.any(0)
                bmax = max(bmax, float(scm.max()))
                rmin = min(rmin, float(rowmax[has].min()))
            # exp(bmax-shift) <= e^4.9 = 134 < 240 (e4m3 max); keep the
            # weakest row's max above the subnormal flush threshold 2^-10
            shift = max(0.0, bmax - 4.9)
            shift = min(shift, rmin + 6.5)
            vm2[:, qb * KC:qb * KC + KC] = vm - shift
        bqk = np.zeros((128, 4), f32)
        bqk[:, 0] = b_q[rows][:128]
        bqk[:, 1] = b_q[rows][128:]
        bqk[:, 2] = b_k[rows][:128]
        bqk[:, 3] = b_k[rows][128:]
        in_maps.append({
            "xqT": t_bf(xq[b]),
            "xkT": t_bf(xk[b][:KP]),
            "xvT": t_bf(xv[b][:KP]),
            "wqT": t_bf(w_q[rows]),
            "wkT": t_bf(w_k[rows]),
            "wvT": t_bf(w_v[rows]),
            "woT": t_bf(w_o[:, rows]),
            "vmask": vm2,
            "bqk": bqk,
        })
    # exact host-side bias correction: y += b_o + w_o @ b_v
    host_bias = (b_o + w_o @ b_v).astype(f32)
    return in_maps, KC, host_bias


def combine_outputs(results, host_bias):
    B = 2
    out = np.zeros((B, S, D), np.float32)
    for core, res in enumerate(results):
        b = core // 4
        out[b] += np.asarray(res["y"], np.float32)
    out += host_bias[None, None, :]
    return out


# ---------------------------------------------------------------------------
# Harness entry point: full (unsharded) inputs -> full output.
# Shards across the 8 NeuronCores as 2 batches x 4 head-groups, runs the
# Bass kernel SPMD, and reduces the per-core partial outputs on the host.
# ---------------------------------------------------------------------------
def kernel(**inputs) -> np.ndarray:
    from concourse.bass_utils import run_bass_kernel_spmd

    in_maps, KC, host_bias = prepare_inputs(inputs)
    nc = build_kernel(KC, {"attn_ratio": 4})
    res = run_bass_kernel_spmd(nc, in_maps, list(range(8)))
    return combine_outputs(res.results, host_bias)


# revision 21
# speedup vs baseline: 1.1048x; 1.0070x over previous
"""Multi-head attention (sparse/causal+valid_len) Bass kernel for TRN2.

Sharding: 8 cores = 2 batches x 4 head-groups (4 heads each).
Each core: this batch's x-tensors + its head-group's weight slices,
computes a partial (S, D_MODEL) output (its heads' contribution through
w_o); host sums the 4 partials per batch and adds biases.

Layouts (bf16 compute, fp32 accumulate in PSUM):
  qT, kT  [head_dim(2 heads=128), S]  - transposed projections
  v4      [k, h, (hi,lo), 65] fp8e4   - exact fp8 pair + ones col (denom)
  scoresT [k, q] in PSUM -> exp on ACT (scale=1/8, bias=vmask-shift) -> fp8
  causal zeroing of diagonal tiles via gpsimd affine_select
  attnV   fp8 DoubleRow: dup(exp) x (v_hi, v_lo) -> po4 [q, h, 65] fp32
  normalize: one scalar_tensor_tensor with broadcast reciprocals -> att bf16
  PE-transpose att -> attn_oT [hd, q] -> out-proj vs woT -> y fp16

The exp range shifts (per core, per 512-query block) keep exp() inside
fp8e4m3 dynamic range; softmax normalization cancels them exactly.
"""

import numpy as np
import ml_dtypes

import concourse.bass as bass
import concourse.mybir as mybir
import concourse.tile as tile
from concourse.masks import make_identity

BF16 = mybir.dt.bfloat16
FP16 = mybir.dt.float16
FP32 = mybir.dt.float32
FP8 = mybir.dt.float8e4

S = 2048
D = 1024
HEADS_PER_CORE = 4   # head-group size
DH = 64
HD = HEADS_PER_CORE * DH          # 256
NEG = -1.0e5                      # additive mask; exp underflows to exactly 0
NQB = S // 512

_MAX_WAITS = 1  # this container's walrus allows 1 sync wait per instruction

DR = mybir.MatmulPerfMode.DoubleRow


def fix_multi_waits(nc, max_waits: int = _MAX_WAITS):
    """Split >max_waits sem waits onto EventSemaphore insts placed just
    before the owning instruction (same engine => same semantics)."""
    import bass_rust
    n = 0
    for f in nc.m.functions:
        for bb in f.blocks:
            out = []
            changed = False
            for ins in bb.instructions:
                si = ins.sync_info
                waits = list(si.on_wait) if si is not None else []
                if len(waits) > max_waits:
                    changed = True
                    extra = waits[:-max_waits]
                    si.on_wait = waits[-max_waits:]
                    for i in range(0, len(extra), max_waits):
                        n += 1
                        es = mybir.InstEventSemaphore(
                            name=f"{ins.name}-esw{i}", ins=[], outs=[])
                        es.engine = ins.engine
                        es.sync_info = bass_rust.SyncInfo(
                            on_wait=extra[i:i + max_waits], on_update=[])
                        out.append(es)
                out.append(ins)
            if changed:
                bb.instructions = out
    return n


def build_kernel(KC: int, opts=None):
    opts = dict(opts or {})
    BIG_KC = KC > 8   # large valid_len: keep SBUF in budget
    EXP_BUFS = (2 * KC) if BIG_KC else (4 * KC + 2)
    PSCORE = opts.get("pscore", 2)
    PYO = opts.get("pyo", 2)
    PSMALL = opts.get("psmall", 2)
    ATTN_RATIO = opts.get("attn_ratio", 2)
    Y_ENG = opts.get("y_eng", "dve")
    """Build the per-core Bass program. KC = number of 128-wide key chunks."""
    KP = KC * 128
    NQT = S // 128     # 16 query tiles of 128
    DM = D // 128      # 8 contraction chunks

    nc = bass.Bass()

    # DRAM I/O (per-core values supplied via in_maps)
    xqT_d = nc.dram_tensor("xqT", [D, S], BF16, kind="ExternalInput")
    xkT_d = nc.dram_tensor("xkT", [D, KP], BF16, kind="ExternalInput")
    xvT_d = nc.dram_tensor("xvT", [D, KP], BF16, kind="ExternalInput")
    wqT_d = nc.dram_tensor("wqT", [D, HD], BF16, kind="ExternalInput")
    wkT_d = nc.dram_tensor("wkT", [D, HD], BF16, kind="ExternalInput")
    wvT_d = nc.dram_tensor("wvT", [D, HD], BF16, kind="ExternalInput")
    woT_d = nc.dram_tensor("woT", [HD, D], BF16, kind="ExternalInput")
    vmask_d = nc.dram_tensor("vmask", [128, NQB * KC], FP32,
                             kind="ExternalInput")
    bqk_d = nc.dram_tensor("bqk", [128, 4], FP32, kind="ExternalInput")
    y_d = nc.dram_tensor("y", [S, D], FP16, kind="ExternalOutput")

    with tile.TileContext(nc) as tc:
        with (
            tc.tile_pool(name="const", bufs=1) as cpool,
            tc.tile_pool(name="win", bufs=1) as wpool,
            tc.tile_pool(name="qkv", bufs=1) as qkvpool,
            tc.tile_pool(name="ao", bufs=4) as aopool,
            tc.tile_pool(name="ysb", bufs=3) as ypool,
            tc.tile_pool(name="ps_score", bufs=PSCORE, space="PSUM") as pscore,
            tc.tile_pool(name="ps_yo", bufs=PYO, space="PSUM") as pyo,
            tc.tile_pool(name="ps_small", bufs=PSMALL, space="PSUM") as psmall,
        ):
            from contextlib import ExitStack
            xstack = ExitStack()
            xpool = xstack.enter_context(tc.tile_pool(name="xin", bufs=1))
            estack = ExitStack()
            epool = None
            if not BIG_KC:
                epool = estack.enter_context(
                    tc.tile_pool(name="expp", bufs=EXP_BUFS))
            # ---- constants (tiny, needed early) ----
            ident = cpool.tile([128, 128], BF16, tag="ident")
            make_identity(nc, ident[:, :])
            # ---- loads, ordered so scores(qb0) unblocks ASAP; kproj is
            # emitted c-outermost so PE chews each xk chunk as it lands ----
            wkT = wpool.tile([128, DM, HD], BF16, tag="wkT")
            wk_r = wkT_d[:].rearrange("(c p) f -> p c f", p=128)
            nc.sync.dma_start(wkT[:, 0:2, :], wk_r[:, 0:2, :])
            xkT = xpool.tile([128, DM, KP], BF16, tag="xkT")
            xk_r = xkT_d[:].rearrange("(c p) f -> p c f", p=128)
            nc.sync.dma_start(xkT[:, 0:2, :], xk_r[:, 0:2, :])
            bqk = cpool.tile([128, 4], FP32, tag="bqk")
            nc.sync.dma_start(bqk[:, :], bqk_d[:, :])
            vmask = cpool.tile([128, NQB * KC], FP32, tag="vmask")
            nc.sync.dma_start(vmask[:, :], vmask_d[:, :])
            # q-side of block 0 loads before the remaining k chunks: qproj(0)
            # fills the PE while the rest of xk streams in
            wqT = wpool.tile([128, DM, HD], BF16, tag="wqT")
            nc.sync.dma_start(
                wqT[:, :, :], wqT_d[:].rearrange("(c p) f -> p c f", p=128))
            xqT = xpool.tile([128, DM, S], BF16, tag="xqT")
            xq_r = xqT_d[:].rearrange("(c p) f -> p c f", p=128)
            nc.sync.dma_start(xqT[:, :, 0:512], xq_r[:, :, 0:512])
            nc.sync.dma_start(wkT[:, 2:DM, :], wk_r[:, 2:DM, :])
            nc.sync.dma_start(xkT[:, 2:DM, :], xk_r[:, 2:DM, :])
            wvT = wpool.tile([128, DM, HD], BF16, tag="wvT")
            nc.sync.dma_start(
                wvT[:, :, :], wvT_d[:].rearrange("(c p) f -> p c f", p=128))
            xvT = xpool.tile([128, DM, KP], BF16, tag="xvT")
            xv_r = xvT_d[:].rearrange("(c p) f -> p c f", p=128)
            for c in range(0, DM, 4):
                nc.sync.dma_start(xvT[:, c:c + 4, :], xv_r[:, c:c + 4, :])
            for qs in range(512, S, 512):
                nc.sync.dma_start(
                    xqT[:, :, qs:qs + 512], xq_r[:, :, qs:qs + 512])
            woT = wpool.tile([128, 2, D], BF16, tag="woT")
            nc.sync.dma_start(
                woT[:, :, :], woT_d[:].rearrange("(c p) f -> p c f", p=128))

            # ---- K projection, c-outermost: accumulates each x-chunk as it
            # arrives; both ks-blocks' psum groups stay open in pscore ----
            kT = [qkvpool.tile([128, KP], BF16, tag=f"kT{j}", name=f"kT{j}")
                  for j in range(2)]
            # c-outermost: accumulates each x-chunk as it arrives; all
            # (block, j) psum groups stay open in pscore (distinct banks).
            # Emitted in two parts with qproj(0) between so the PE has work
            # while the trailing xk chunks stream in.
            kblocks = [(ks, min(512, KP - ks)) for ks in range(0, KP, 512)]
            kps = [pscore.tile([128, 2, 512], FP32, tag="pssc",
                               name=f"kps{i}") for i in range(len(kblocks))]

            def emit_kproj(cs):
                for c in cs:
                    for i, (ks, w) in enumerate(kblocks):
                        for j in range(2):
                            nc.tensor.matmul(
                                kps[i][:, j, :w],
                                wkT[:, c, 128 * j:128 * j + 128],
                                xkT[:, c, ks:ks + w],
                                start=(c == 0), stop=(c == DM - 1))
                if cs[-1] == DM - 1:
                    for i, (ks, w) in enumerate(kblocks):
                        for j in range(2):
                            nc.vector.tensor_scalar_add(
                                kT[j][:, ks:ks + w], kps[i][:, j, :w],
                                bqk[:, 2 + j:3 + j])

            # ---- V projection: generator, interleaved during qb0 scores.
            # Output as exact fp8 (hi, lo) pair + ones column (denominator).
            FP8_ATTN = opts.get("fp8_attn", False)
            ET_DT = FP8 if FP8_ATTN else BF16
            if FP8_ATTN:
                v_t = [qkvpool.tile([128, HEADS_PER_CORE, 2, 65], FP8,
                                    tag=f"v{kb}", name=f"v{kb}")
                       for kb in range(KC)]
            else:
                v_t = [qkvpool.tile([128, HEADS_PER_CORE, 65], BF16,
                                    tag=f"v{kb}", name=f"v{kb}")
                       for kb in range(KC)]

            def emit_vproj():
                for kb in range(KC):
                    vt = v_t[kb]
                    ps = pyo.tile([128, 512], FP32, tag="psy",
                                  name=f"psv{kb}")
                    for c in range(DM):
                        nc.tensor.matmul(
                            ps[:, :HD],
                            xvT[:, c, 128 * kb:128 * kb + 128],
                            wvT[:, c, :],
                            start=(c == 0), stop=(c == DM - 1))
                    psh = ps[:, :HD].rearrange("p (h e) -> p h e", e=64)
                    if FP8_ATTN:
                        nc.scalar.copy(vt[:, :, 0, 0:64], psh)
                        nc.vector.tensor_sub(vt[:, :, 1, 0:64], psh,
                                             vt[:, :, 0, 0:64])
                        nc.gpsimd.memset(vt[:, :, 0, 64:65], 1.0)
                        nc.gpsimd.memset(vt[:, :, 1, 64:65], 0.0)
                    else:
                        nc.vector.tensor_copy(vt[:, :, 0:64], psh)
                        nc.gpsimd.memset(vt[:, :, 64:65], 1.0)
                    yield

            # ---- Q projection (emitted per query block, pipelined) ----
            qT = [qkvpool.tile([128, S], BF16, tag=f"qT{j}", name=f"qT{j}")
                  for j in range(2)]

            def emit_qproj(qb):
                qs = 512 * qb
                for j in range(2):
                    ps = pyo.tile([128, 512], FP32, tag="psy",
                                  name=f"psq{qb}_{j}")
                    for c in range(DM):
                        nc.tensor.matmul(
                            ps[:, :],
                            wqT[:, c, 128 * j:128 * j + 128],
                            xqT[:, c, qs:qs + 512],
                            start=(c == 0), stop=(c == DM - 1))
                    nc.vector.tensor_scalar_add(
                        qT[j][:, qs:qs + 512], ps[:, :], bqk[:, j:j + 1])
                    yield

            # ---- attention + output projection, per 512-query block ----
            # software-pipelined: scores/exp for qb+1 are emitted before
            # attnV/outproj of qb so PE never waits on ACT's exp pass
            attn_oT = qkvpool.tile([128, 2, S], BF16, tag="aoT", name="aoT")
            exp_stage = {}
            epool_ref = [None]

            def emit_scores(qb):
                # generator: yields after each (kt, pair) score unit
                ktm = min(4 * qb + 3, KC - 1)   # causal+valid key-chunk bound
                # scoresT [k, q] -> exp (with per-qb range shift) -> fp8
                expT = [[None] * (ktm + 1) for _ in range(HEADS_PER_CORE)]
                exp_qlo = [0] * (ktm + 1)
                exp_stage[qb] = (expT, exp_qlo)
                for kt in range(ktm + 1):
                    for j in range(2):
                        # causal: queries below 128*kt never see this k chunk
                        qlo = max(0, 128 * kt - 512 * qb)
                        exp_qlo[kt] = qlo
                        w = 512 - qlo
                        # both row-halves (heads 2j, 2j+1) share one psum
                        # tile (different banks -> still concurrent on PE)
                        # and one exp + one causal-select instruction
                        ps = pscore.tile([128, 2, 512], FP32, tag="pssc",
                                         name=f"pssc{qb}_{kt}_{j}")
                        for r in range(2):
                            nc.tensor.matmul(
                                ps[:, r, :w],
                                kT[j][64 * r:64 * r + 64,
                                      128 * kt:128 * kt + 128],
                                qT[j][64 * r:64 * r + 64,
                                      512 * qb + qlo:512 * qb + 512],
                                start=True, stop=True)
                        et = epool_ref[0].tile([128, 2, w], ET_DT, tag="expT",
                                               name=f"expT{qb}_{kt}_{j}")
                        nc.scalar.activation(
                            et[:, :, :], ps[:, :, :w],
                            mybir.ActivationFunctionType.Exp,
                            bias=vmask[:, qb * KC + kt:qb * KC + kt + 1],
                            scale=0.125)
                        if 128 * kt + 127 > 512 * qb + qlo:
                            # zero strictly-above-diagonal: keep q >= k
                            # (r-dim coefficient 0: same mask per head)
                            nc.gpsimd.affine_select(
                                out=et[:, :, :], in_=et[:, :, :],
                                compare_op=mybir.AluOpType.is_ge,
                                fill=0.0,
                                base=512 * qb + qlo - 128 * kt,
                                pattern=[[0, 2], [1, w]],
                                channel_multiplier=-1)
                        expT[2 * j][kt] = et
                        expT[2 * j + 1][kt] = et
                        yield

            # 2-deep skewed attention pipeline: PE order per step is
            # attnV(qt), transposes(qt-1), outproj(qt-2) so DVE/ACT
            # evacuations of a tile overlap PE work on its neighbours
            stage_t = []   # (qt, att) awaiting transpose
            stage_o = []   # qt awaiting out-projection

            def flush_transpose():
                qt, att = stage_t.pop(0)
                pst = psmall.tile([128, 2, 128], BF16, tag="pso",
                                  name=f"pst{qt}")
                for j in range(2):
                    nc.tensor.matmul(
                        pst[:, j, :], att[:, 128 * j:128 * j + 128],
                        ident[:, :], is_transpose=True,
                        start=(j == 0), stop=(j == 1),
                        skip_group_check=True)
                nc.vector.tensor_copy(
                    attn_oT[:, :, 128 * qt:128 * qt + 128], pst[:, :, :])
                stage_o.append(qt)

            def flush_outproj():
                qt = stage_o.pop(0)
                ys = ypool.tile([128, D], FP16, tag="ysb")
                for n in range(2):
                    ps = pyo.tile([128, 512], FP32, tag="psy")
                    for hc in range(2):
                        nc.tensor.matmul(
                            ps[:, :],
                            attn_oT[:, hc, 128 * qt:128 * qt + 128],
                            woT[:, hc, 512 * n:512 * n + 512],
                            start=(hc == 0), stop=(hc == 1))
                    # split PSUM evacuation between ACT and DVE
                    if Y_ENG == "act" or (Y_ENG == "split" and n == 0):
                        nc.scalar.copy(ys[:, 512 * n:512 * n + 512],
                                       ps[:, :])
                    else:
                        nc.vector.tensor_copy(
                            ys[:, 512 * n:512 * n + 512], ps[:, :])
                    nc.sync.dma_start(
                        y_d[128 * qt:128 * qt + 128,
                            512 * n:512 * n + 512],
                        ys[:, 512 * n:512 * n + 512])

            def emit_attn(qb):
                expT, exp_qlo = exp_stage.pop(qb)
                for qq in range(4):             # 128-query tiles in this block
                    qt = 4 * qb + qq
                    ktm_q = min(qt, KC - 1)
                    # all 4 heads share one psum bank: a single accumulation
                    # group (start zeroes the whole 2KB zero-region once)
                    po4 = psmall.tile([128, HEADS_PER_CORE, 65], FP32,
                                      tag="pso", name=f"po{qt}")
                    for h in range(HEADS_PER_CORE):
                        for kt in range(ktm_q + 1):
                            c0 = 128 * qq - exp_qlo[kt]
                            st = (h == 0 and kt == 0)
                            sp = (h == HEADS_PER_CORE - 1 and kt == ktm_q)
                            use_dr = opts.get("dr", True)
                            if isinstance(use_dr, list):
                                use_dr = qb in use_dr
                            if FP8_ATTN and use_dr:
                                lhsT = (expT[h][kt][:, h % 2, c0:c0 + 128]
                                        .unsqueeze(1)
                                        .broadcast_to([128, 2, 128]))
                                nc.tensor.matmul(
                                    po4[:, h, :], lhsT, v_t[kt][:, h, :, :],
                                    start=st, stop=sp,
                                    perf_mode=DR, skip_group_check=True)
                            elif FP8_ATTN:
                                es = expT[h][kt][:, h % 2, c0:c0 + 128]
                                nc.tensor.matmul(
                                    po4[:, h, :], es, v_t[kt][:, h, 0, :],
                                    start=st, stop=False,
                                    skip_group_check=True)
                                nc.tensor.matmul(
                                    po4[:, h, :], es, v_t[kt][:, h, 1, :],
                                    start=False, stop=sp,
                                    skip_group_check=True)
                            else:
                                nc.tensor.matmul(
                                    po4[:, h, :],
                                    expT[h][kt][:, h % 2, c0:c0 + 128],
                                    v_t[kt][:, h, :],
                                    start=st, stop=sp, skip_group_check=True)
                        yield
                    rec4 = aopool.tile([128, HEADS_PER_CORE], FP32, tag="rec")
                    for h in range(HEADS_PER_CORE):
                        nc.vector.reciprocal(rec4[:, h:h + 1],
                                             po4[:, h, 64:65])
                    att = aopool.tile([128, HD], BF16, tag="att",
                                      name=f"att{qt}")
                    nc.vector.scalar_tensor_tensor(
                        att[:].rearrange("p (h e) -> p h e", e=64),
                        po4[:, :, 0:64], 1.0,
                        rec4[:].unsqueeze(2).broadcast_to(
                            [128, HEADS_PER_CORE, 64]),
                        op0=mybir.AluOpType.mult, op1=mybir.AluOpType.mult)
                    stage_t.append((qt, att))
                    if len(stage_t) > 1:
                        flush_transpose()
                    if len(stage_o) > 1:
                        flush_outproj()
                    yield

            if epool is not None:
                epool_ref[0] = epool
            # kproj(c0-1) -> qproj(0) -> kproj(c2-7): the PE chews early
            # chunks while the rest of xk/xq streams in
            emit_kproj(list(range(0, 2)))
            for _ in emit_qproj(0):
                pass
            emit_kproj(list(range(2, DM)))
            vp = emit_vproj()
            if BIG_KC:
                # all projections upfront, then release x inputs from SBUF
                # and only then open the (large) exp pool in the freed zone
                for _ in vp:
                    pass
                for qb_ in range(1, NQB):
                    for _ in emit_qproj(qb_):
                        pass
                vp = None
                xstack.close()
                epool_ref[0] = estack.enter_context(
                    tc.tile_pool(name="expp", bufs=EXP_BUFS))
            for qb in range(NQB + 1):
                sc = emit_scores(qb) if qb < NQB else None
                at = (emit_attn(qb - 1)
                      if qb >= 1 and qb - 1 in exp_stage else None)
                qp = (emit_qproj(qb + 1)
                      if (not BIG_KC and qb + 1 < NQB) else None)
                done_sc = sc is None
                done_at = at is None
                done_qp = qp is None
                if qb == 0 and vp is not None:
                    at, done_at = vp, False
                    vp = None
                while not (done_sc and done_at and done_qp):
                    if not done_sc:
                        try:
                            next(sc)
                        except StopIteration:
                            done_sc = True
                    if not done_at:
                        for _ in range(ATTN_RATIO):
                            try:
                                next(at)
                            except StopIteration:
                                done_at = True
                                break
                    if not done_qp:
                        try:
                            next(qp)
                        except StopIteration:
                            done_qp = True

            # drain the skewed attention pipeline
            while stage_t or stage_o:
                if stage_t:
                    flush_transpose()
                if stage_o:
                    flush_outproj()

            estack.close()
            if not BIG_KC:
                xstack.close()

    fix_multi_waits(nc)
    return nc


def prepare_inputs(inputs):
    """Host-side shard/cast/transpose. Returns (in_maps, KC, host_bias)."""
    f32 = np.float32
    xq = np.asarray(inputs["will_be_queries"], f32)
    xk = np.asarray(inputs["will_be_keys"], f32)
    xv = np.asarray(inputs["will_be_values"], f32)
    L = np.asarray(inputs["valid_len"]).astype(np.int64)
    w_q = np.asarray(inputs["w_q"], f32)
    w_k = np.asarray(inputs["w_k"], f32)
    w_v = np.asarray(inputs["w_v"], f32)
    w_o = np.asarray(inputs["w_o"], f32)
    b_q = np.asarray(inputs["b_q"], f32)
    b_k = np.asarray(inputs["b_k"], f32)
    b_o = np.asarray(inputs["b_o"], f32)
    b_v = np.asarray(inputs["b_v"], f32)

    B = xq.shape[0]
    Lmax = int(L.max())
    KC = (Lmax + 127) // 128
    KP = KC * 128
    bf = ml_dtypes.bfloat16

    def t_bf(a):  # (r, c) -> transposed bf16 contiguous
        return np.ascontiguousarray(a.T).astype(bf)

    bf16 = ml_dtypes.bfloat16
    in_maps = []
    for core in range(8):
        b, hg = divmod(core, 4)
        rows = slice(HD * hg, HD * hg + HD)
        # exp range shifts per q-block: exact block-max of valid logits
        # (device-identical bf16 q/k) keeps exp() inside fp8e4m3 range;
        # softmax normalization cancels the shift exactly.
        qTc = ((w_q[rows] @ xq[b].T).astype(bf16).astype(f32))  # (HD, S)
        kTc = ((w_k[rows] @ xk[b][:KP].T).astype(bf16).astype(f32))
        k_idx1 = np.arange(KP)[:, None]
        vm = np.full((128, KC), 0.0, f32)
        k_idx = (np.arange(KC)[None, :] * 128 + np.arange(128)[:, None])
        vm[k_idx >= L[b]] = NEG
        vm2 = np.empty((128, NQB * KC), f32)
        for qb in range(NQB):
            bmax, rmin = -1e9, 1e9
            q_idx1 = 512 * qb + np.arange(512)[None, :]
            for h in range(HEADS_PER_CORE):
                sc = (kTc[DH * h:DH * h + DH].T
                      @ qTc[DH * h:DH * h + DH, 512 * qb:512 * qb + 512])
                sc *= 0.125
                valid = (k_idx1 <= q_idx1) & (k_idx1 < L[b])
                scm = np.where(valid, sc, -1e9)
                rowmax = scm.max(0)
                has = valid.any(0)
                bmax = max(bmax, float(scm.max()))
                rmin = min(rmin, float(rowmax[has].min()))
            # exp(bmax-shift) <= e^4.9 = 134 < 240 (e4m3 max); keep the
            # weakest row's max above the subnormal flush threshold 2^-10
            shift = max(0.0, bmax - 4.9)
            shift = min(shift, rmin + 6.5)
            vm2[:, qb * KC:qb * KC + KC] = vm - shift
        bqk = np.zeros((128, 4), f32)
        bqk[:, 0] = b_q[rows][:128]
        bqk[:, 1] = b_q[rows][128:]
        bqk[:, 2] = b_k[rows][:128]
        bqk[:, 3] = b_k[rows][128:]
        in_maps.append({
            "xqT": t_bf(xq[b]),
            "xkT": t_bf(xk[b][:KP]),
            "xvT": t_bf(xv[b][:KP]),
            "wqT": t_bf(w_q[rows]),
            "wkT": t_bf(w_k[rows]),
            "wvT": t_bf(w_v[rows]),
            "woT": t_bf(w_o[:, rows]),
            "vmask": vm2,
            "bqk": bqk,
        })
    # exact host-side bias correction: y += b_o + w_o @ b_v
    host_bias = (b_o + w_o @ b_v).astype(f32)
    return in_maps, KC, host_bias


def combine_outputs(results, host_bias):
    B = 2
    out = np.zeros((B, S, D), np.float32)
    for core, res in enumerate(results):
        b = core // 4
        out[b] += np.asarray(res["y"], np.float32)
    out += host_bias[None, None, :]
    return out


# ---------------------------------------------------------------------------
# Harness entry point: full (unsharded) inputs -> full output.
# Shards across the 8 NeuronCores as 2 batches x 4 head-groups, runs the
# Bass kernel SPMD, and reduces the per-core partial outputs on the host.
# ---------------------------------------------------------------------------
def kernel(**inputs) -> np.ndarray:
    from concourse.bass_utils import run_bass_kernel_spmd

    in_maps, KC, host_bias = prepare_inputs(inputs)
    nc = build_kernel(KC, {"attn_ratio": 4})
    res = run_bass_kernel_spmd(nc, in_maps, list(range(8)))
    return combine_outputs(res.results, host_bias)
